# revision 19
# baseline (speedup 1.0000x reference)
"""Trainium2 Bass kernel for nn_BrainNN (GNN message passing).

Math per sample b (data-parallel: one sample per NeuronCore):
  h   = enc(X[b])                                   # [N, HID] (kept transposed [HID, N])
  for l in 0,1:
    s[i,m] = sum_j relu(a[i,m] + b1[m] + bb[j,m] + ww[m]*W[i,j])
    h     += relu(s @ w2.T + N*b2)
  z' = h.sum(rows); z = ro(z') + z'; y = cls(z)

Device mapping (layout B: m on partitions, j on free axis):
  - PE per i-pair: K=128 matmul re-streams bb into PSUM (rhs = hT repeated
    twice -> F=512 covers 2 i's), then a K=1 matmul accumulates
    ww[m] (x) W[i,:] on top (W rows packed on partitions {0,32,64,96} so a
    [1,512] rhs row slice is 32-aligned).
  - Evacuation alternates between ScalarE and VectorE, one instruction per
    (i, m-tile): ACT relu(psum + bias[m]) with free-axis accum_out, or DVE
    scalar_tensor_tensor max(psum + bias, 0) with accum_out.  accum_out
    columns land directly in sT[m, i].
"""

import numpy as np

import concourse.bacc as bacc
import concourse.bass as bass
import concourse.tile as tile
from concourse import mybir
from concourse.bass_utils import run_bass_kernel_spmd

F32 = mybir.dt.float32
F16 = mybir.dt.float16
OP = mybir.AluOpType
AF = mybir.ActivationFunctionType

B, N, D, HID, MSG, RO, L, C = 8, 256, 256, 128, 256, 128, 2, 2
NCORES = 8


def _ap(t, ap, offset=0):
    return bass.AP(tensor=t.tensor, offset=t.offset + offset, ap=ap)


def build_nc():
    nc = bacc.Bacc("TRN2")

    xt = nc.dram_tensor("xt", [128, 2 * N], F16, kind="ExternalInput")
    wpack = nc.dram_tensor("wpack", [128, 64 * N], F16, kind="ExternalInput")
    encw1t = nc.dram_tensor("encw1t", [128, 2 * HID], F16, kind="ExternalInput")
    encw2t = nc.dram_tensor("encw2t", [128, HID], F16, kind="ExternalInput")
    encb1 = nc.dram_tensor("encb1", [128, 1], F32, kind="ExternalInput")
    encb2 = nc.dram_tensor("encb2", [128, 1], F32, kind="ExternalInput")
    watc = nc.dram_tensor("watc", [128, L * MSG], F16, kind="ExternalInput")
    wbtc = nc.dram_tensor("wbtc", [128, L * MSG], F16, kind="ExternalInput")
    ww4 = nc.dram_tensor("ww4", [128, L * 2 * 4 * 128], F16, kind="ExternalInput")
    msgb1 = nc.dram_tensor("msgb1", [128, L * 2], F32, kind="ExternalInput")
    w2tc = nc.dram_tensor("w2tc", [128, L * MSG], F16, kind="ExternalInput")
    nb2 = nc.dram_tensor("nb2", [128, L], F32, kind="ExternalInput")
    row1t = nc.dram_tensor("row1t", [128, RO], F32, kind="ExternalInput")
    rob1 = nc.dram_tensor("rob1", [128, 1], F32, kind="ExternalInput")
    row2t = nc.dram_tensor("row2t", [128, RO], F32, kind="ExternalInput")
    rob2 = nc.dram_tensor("rob2", [128, 1], F32, kind="ExternalInput")
    clswt = nc.dram_tensor("clswt", [128, C], F32, kind="ExternalInput")
    clsb = nc.dram_tensor("clsb", [C, 1], F32, kind="ExternalInput")
    y = nc.dram_tensor("y", [C, 1], F32, kind="ExternalOutput")

    with tile.TileContext(nc) as tc:
        with (
            tc.tile_pool(name="consts", bufs=1) as consts,
            tc.tile_pool(name="work", bufs=2) as work,
            tc.tile_pool(name="mmp", bufs=1, space="PSUM") as mmp,
            tc.tile_pool(name="aacc", bufs=1, space="PSUM") as aacc,
            tc.tile_pool(name="inner", bufs=6, space="PSUM") as innerp,
        ):
            dma = nc.sync.dma_start

            # ---- constant loads -------------------------------------------
            t_xt = consts.tile([128, 2 * N], F16)
            dma(out=t_xt[:, :], in_=xt[:, :])
            t_encw1t = consts.tile([128, 2 * HID], F16)
            dma(out=t_encw1t[:, :], in_=encw1t[:, :])
            t_encw2t = consts.tile([128, HID], F16)
            dma(out=t_encw2t[:, :], in_=encw2t[:, :])
            t_encb1 = consts.tile([128, 1], F32)
            dma(out=t_encb1[:, :], in_=encb1[:, :])
            t_encb2 = consts.tile([128, 1], F32)
            dma(out=t_encb2[:, :], in_=encb2[:, :])
            t_wat = consts.tile([128, L * MSG], F16)
            dma(out=t_wat[:, :], in_=watc[:, :])
            t_wbt = consts.tile([128, L * MSG], F16)
            dma(out=t_wbt[:, :], in_=wbtc[:, :])
            t_msgb1 = consts.tile([128, L * 2], F32)
            dma(out=t_msgb1[:, :], in_=msgb1[:, :])
            t_w2t = consts.tile([128, L * MSG], F16)
            dma(out=t_w2t[:, :], in_=w2tc[:, :])
            t_nb2 = consts.tile([128, L], F32)
            dma(out=t_nb2[:, :], in_=nb2[:, :])
            t_row1t = consts.tile([128, RO], F32)
            dma(out=t_row1t[:, :], in_=row1t[:, :])
            t_rob1 = consts.tile([128, 1], F32)
            dma(out=t_rob1[:, :], in_=rob1[:, :])
            t_row2t = consts.tile([128, RO], F32)
            dma(out=t_row2t[:, :], in_=row2t[:, :])
            t_rob2 = consts.tile([128, 1], F32)
            dma(out=t_rob2[:, :], in_=rob2[:, :])
            t_clswt = consts.tile([128, C], F32)
            dma(out=t_clswt[:, :], in_=clswt[:, :])
            t_clsb = consts.tile([C, 1], F32)
            dma(out=t_clsb[:, :], in_=clsb[:, :])


            # W packed dense: 8 column chunks, alternating HWDGE/SWDGE
            # queues so transfers overlap; first pairs need only chunk 0.
            t_wpack = consts.tile([128, 64 * N], F16)
            t_ww4 = consts.tile([128, L * 2 * 4 * 128], F16)
            dma(out=t_ww4[:, :], in_=ww4[:, :])
            CH = 64 * N // 8
            for c in range(8):
                eng = dma if c % 2 == 0 else nc.gpsimd.dma_start
                eng(out=t_wpack[:, c * CH:(c + 1) * CH],
                    in_=wpack[:, c * CH:(c + 1) * CH])

            # zero tile for the DVE max() path
            t_zero = consts.tile([128, N], F32)
            nc.vector.memset(t_zero[:, :], 0.0)

            # ---- encoder ---------------------------------------------------
            pe1 = mmp.tile([128, N], F32, tag="mm", name="pe1")
            nc.tensor.matmul(pe1[:, :], t_encw1t[:, 0:128], t_xt[:, 0:N],
                             start=True, stop=False)
            nc.tensor.matmul(pe1[:, :], t_encw1t[:, 128:256], t_xt[:, N:2 * N],
                             start=False, stop=True)
            t_h1 = work.tile([128, N], F16, tag="h1")
            nc.scalar.activation(out=t_h1[:, :], in_=pe1[:, :], func=AF.Relu,
                                 bias=t_encb1[:, 0:1])
            pe2 = mmp.tile([128, N], F32, tag="mm", name="pe2")
            nc.tensor.matmul(pe2[:, :], t_encw2t[:, :], t_h1[:, :],
                             start=True, stop=True)
            t_h = work.tile([128, N], F32, tag="hT")
            nc.vector.tensor_scalar(out=t_h[:, :], in0=pe2[:, :],
                                    scalar1=t_encb2[:, 0:1], scalar2=None,
                                    op0=OP.add)
            t_h16 = work.tile([128, N], F16, tag="hT16")
            nc.vector.tensor_copy(t_h16[:, :], t_h[:, :])

            # ---- message-passing layers -----------------------------------
            for l in range(L):
                # aT (+ msg_b1) per m-tile: bias columns for the inner loop
                t_ab = []
                for mt in range(2):
                    pa = mmp.tile([128, N], F32, tag="mm", name="pa")
                    nc.tensor.matmul(
                        pa[:, :],
                        t_wat[:, l * MSG + mt * 128: l * MSG + (mt + 1) * 128],
                        t_h16[:, :], start=True, stop=True)
                    ab = work.tile([128, N], F32, tag=f"ab{mt}")
                    nc.vector.tensor_scalar(
                        out=ab[:, :], in0=pa[:, :],
                        scalar1=t_msgb1[:, (l * 2 + mt):(l * 2 + mt) + 1],
                        scalar2=None, op0=OP.add)
                    nab = work.tile([128, N], F32, tag=f"nab{mt}", name=f"nab{mt}")
                    nc.vector.tensor_scalar(
                        out=nab[:, :], in0=pa[:, :],
                        scalar1=t_msgb1[:, (l * 2 + mt):(l * 2 + mt) + 1],
                        scalar2=-1.0, op0=OP.add, op1=OP.mult)
                    t_ab.append((ab, nab))

                t_sp = work.tile([128, 2 * N], F32, tag="sTp", name="sTp")
                t_sa = aacc.tile([128, 2 * N], F32, tag="sTa", name="sTa")
                t_s = [t_sp[:, mt * N:(mt + 1) * N] for mt in range(2)]

                # hT repeated twice along free axis (F=512 -> two i's)
                h_ap = t_h16[:, :]
                h_rep = _ap(h_ap, [h_ap.ap[0], [0, 2], h_ap.ap[1]])

                for mt in range(2):
                    wb_sl = t_wbt[:, l * MSG + mt * 128: l * MSG + (mt + 1) * 128]
                    for p in range(N // 2):
                        i0 = 2 * p
                        G = i0 // 64
                        blk = ((l * 2 + mt) * 4 + G) * 128
                        ofs = (i0 % 64) * N
                        ps = innerp.tile([128, 2 * N], F32)
                        nc.tensor.matmul(ps[:, :], wb_sl, h_rep,
                                         start=True, stop=False)
                        nc.tensor.matmul(
                            ps[:, :],
                            t_ww4[:, blk:blk + 128],
                            t_wpack[:, ofs:ofs + 2 * N],
                            start=False, stop=True)
                        for half in range(2):
                            i = i0 + half
                            sl = ps[:, half * N:(half + 1) * N]
                            if p % 32 < 15:
                                nc.scalar.activation(
                                    out=sl, in_=sl, func=AF.Relu,
                                    bias=t_ab[mt][0][:, i:i + 1],
                                    accum_out=t_sa[:, mt * N + i:mt * N + i + 1])
                            else:
                                # accum = sum_j max(pre-c, -c) = s - N*c;
                                # fixed after the loop on the DVE columns
                                nc.vector.tensor_scalar(
                                    out=sl, in0=sl,
                                    scalar1=t_ab[mt][1][:, i:i + 1],
                                    scalar2=None, op0=OP.max, op1=OP.add,
                                    accum_out=t_sp[:, mt * N + i:mt * N + i + 1])

                # message MLP second linear + relu + residual
                t_s16 = [work.tile([128, N], F16, tag=f"sT16_{mt}", name=f"sT16_{mt}")
                         for mt in range(2)]
                for mt in range(2):
                    base = t_sp[:, mt * N:(mt + 1) * N]
                    dcols = _ap(base, [base.ap[0], [64, N // 64], [1, 34]],
                                offset=30)
                    nabd = _ap(t_ab[mt][1][:, :],
                               [t_ab[mt][1][:, :].ap[0], [64, N // 64], [1, 34]],
                               offset=30)
                    s16 = t_s16[mt][:, :]
                    d16 = _ap(s16, [s16.ap[0], [64, N // 64], [1, 34]], offset=30)
                    nc.vector.scalar_tensor_tensor(
                        out=d16, in0=nabd, scalar=-float(N), in1=dcols,
                        op0=OP.mult, op1=OP.add)
                    a16 = _ap(s16, [s16.ap[0], [64, N // 64], [1, 30]], offset=0)
                    abase = t_sa[:, mt * N:(mt + 1) * N]
                    acols = _ap(abase, [abase.ap[0], [64, N // 64], [1, 30]],
                                offset=0)
                    nc.vector.tensor_copy(a16, acols)
                pm = mmp.tile([128, N], F32, tag="mm", name="pm")
                nc.tensor.matmul(pm[:, :], t_w2t[:, (l * 2) * 128:(l * 2 + 1) * 128],
                                 t_s16[0][:, :], start=True, stop=False)
                nc.tensor.matmul(pm[:, :], t_w2t[:, (l * 2 + 1) * 128:(l * 2 + 2) * 128],
                                 t_s16[1][:, :], start=False, stop=True)
                t_md = work.tile([128, N], F32, tag="md")
                nc.scalar.activation(out=t_md[:, :], in_=pm[:, :], func=AF.Relu,
                                     bias=t_nb2[:, l:l + 1])
                t_hn = work.tile([128, N], F32, tag="hT")
                nc.vector.tensor_tensor(out=t_hn[:, :], in0=t_h[:, :],
                                        in1=t_md[:, :], op=OP.add)
                t_h = t_hn
                if l + 1 < L:
                    t_h16 = work.tile([128, N], F16, tag="hT16", name="h16n")
                    nc.vector.tensor_copy(t_h16[:, :], t_h[:, :])

            # ---- readout + classifier -------------------------------------
            t_scr = work.tile([128, N], F32, tag="scr")
            t_z = work.tile([128, 1], F32, tag="zcol")
            nc.scalar.activation(out=t_scr[:, :], in_=t_h[:, :], func=AF.Copy,
                                 accum_out=t_z[:, 0:1])
            pr1 = mmp.tile([128, 1], F32, tag="mm", name="pr1")
            nc.tensor.matmul(pr1[:, :], t_row1t[:, :], t_z[:, 0:1],
                             start=True, stop=True)
            t_r1 = work.tile([128, 1], F32, tag="r1")
            nc.scalar.activation(out=t_r1[:, :], in_=pr1[:, :], func=AF.Relu,
                                 bias=t_rob1[:, 0:1])
            pr2 = mmp.tile([128, 1], F32, tag="mm", name="pr2")
            nc.tensor.matmul(pr2[:, :], t_row2t[:, :], t_r1[:, 0:1],
                             start=True, stop=True)
            t_zf = work.tile([128, 1], F32, tag="zf")
            nc.vector.scalar_tensor_tensor(out=t_zf[:, :], in0=pr2[:, :],
                                           scalar=t_rob2[:, 0:1],
                                           in1=t_z[:, 0:1],
                                           op0=OP.add, op1=OP.add)
            py = mmp.tile([C, 1], F32, tag="mm", name="py")
            nc.tensor.matmul(py[:, :], t_clswt[:, :], t_zf[:, 0:1],
                             start=True, stop=True)
            t_y = work.tile([C, 1], F32, tag="ysb")
            nc.vector.tensor_scalar(out=t_y[:, :], in0=py[:, :],
                                    scalar1=t_clsb[:, 0:1], scalar2=None,
                                    op0=OP.add)
            dma(out=y[:, :], in_=t_y[:, :])

    nc.finalize()
    return nc


def _prep_shared(params):
    """Host-side layout prep of the (tiny) shared parameter tensors."""
    f = np.float32
    enc_w1, enc_b1, enc_w2, enc_b2 = (params["enc_w1"], params["enc_b1"],
                                      params["enc_w2"], params["enc_b2"])
    msg_w1, msg_b1, msg_w2, msg_b2 = (params["msg_w1"], params["msg_b1"],
                                      params["msg_w2"], params["msg_b2"])
    ro_w1, ro_b1, ro_w2, ro_b2 = (params["ro_w1"], params["ro_b1"],
                                  params["ro_w2"], params["ro_b2"])
    cls_w, cls_b = params["cls_w"], params["cls_b"]

    w1t = np.ascontiguousarray(enc_w1.T)          # [D, HID]
    encw1t = np.concatenate([w1t[:128], w1t[128:]], axis=1)  # [128, 256]
    h = np.float16

    watc = np.zeros((128, L * MSG), h)
    wbtc = np.zeros((128, L * MSG), h)
    ww4 = np.zeros((128, L * 2 * 4 * 128), h)
    msgb1c = np.zeros((128, L * 2), f)
    w2tc = np.zeros((128, L * MSG), h)
    nb2c = np.zeros((128, L), f)
    for l in range(L):
        wa = msg_w1[l][:, :HID]                   # [MSG, HID]
        wb = msg_w1[l][:, HID:2 * HID]
        ww = msg_w1[l][:, -1]                     # [MSG]
        watc[:, l * MSG:(l + 1) * MSG] = wa.T
        wbtc[:, l * MSG:(l + 1) * MSG] = wb.T

        w2t = msg_w2[l].T                         # [MSG, HID]
        for mt in range(2):
            msgb1c[:, l * 2 + mt] = msg_b1[l][mt * 128:(mt + 1) * 128]
            for G in range(4):
                blk = ((l * 2 + mt) * 4 + G) * 128
                ww4[32 * G, blk:blk + 128] = ww[mt * 128:(mt + 1) * 128]
            w2tc[:, (l * 2 + mt) * 128:(l * 2 + mt + 1) * 128] = \
                w2t[mt * 128:(mt + 1) * 128]
        nb2c[:, l] = np.float32(N) * msg_b2[l]

    return {
        "encw1t": np.ascontiguousarray(encw1t, h),
        "encw2t": np.ascontiguousarray(enc_w2.T, h),
        "encb1": np.ascontiguousarray(enc_b1.reshape(128, 1), f),
        "encb2": np.ascontiguousarray(enc_b2.reshape(128, 1), f),
        "watc": watc, "wbtc": wbtc, "ww4": ww4, "msgb1": msgb1c,
        "w2tc": w2tc, "nb2": nb2c,
        "row1t": np.ascontiguousarray(ro_w1.T, f),
        "rob1": np.ascontiguousarray(ro_b1.reshape(RO, 1), f),
        "row2t": np.ascontiguousarray(ro_w2.T, f),
        "rob2": np.ascontiguousarray(ro_b2.reshape(RO, 1), f),
        "clswt": np.ascontiguousarray(cls_w.T, f),
        "clsb": np.ascontiguousarray(cls_b.reshape(C, 1), f),
    }


_NC_CACHE = {}


def _get_nc():
    if "nc" not in _NC_CACHE:
        _NC_CACHE["nc"] = build_nc()
    return _NC_CACHE["nc"]


def run(inputs, trace=False):
    inputs = {k: np.asarray(v, dtype=np.float32) for k, v in inputs.items()}
    shared = _prep_shared(inputs)
    W, X = inputs["W"], inputs["X"]
    in_maps = []
    for b in range(NCORES):
        xt = np.ascontiguousarray(X[b].T)         # [D, N]
        m = dict(shared)
        m["xt"] = np.ascontiguousarray(
            np.concatenate([xt[:128], xt[128:]], axis=1), np.float16)
        base = np.float16(W[b]).reshape(4, 64 * N)
        m["wpack"] = np.ascontiguousarray(np.repeat(base, 32, axis=0))
        in_maps.append(m)
    nc = _get_nc()
    res = run_bass_kernel_spmd(nc, in_maps, core_ids=list(range(NCORES)),
                               trace=trace)
    out = np.stack([res.results[b]["y"].reshape(C) for b in range(NCORES)])
    return np.ascontiguousarray(out, np.float32), res


def kernel(**inputs):
    out, _ = run(inputs)
    return out


# revision 20
# speedup vs baseline: 1.0498x; 1.0498x over previous
"""Trainium2 Bass kernel for nn_BrainNN (GNN message passing).

Math per sample b (data-parallel: one sample per NeuronCore):
  h   = enc(X[b])                                   # [N, HID] (kept transposed [HID, N])
  for l in 0,1:
    s[i,m] = sum_j relu(a[i,m] + b1[m] + bb[j,m] + ww[m]*W[i,j])
    h     += relu(s @ w2.T + N*b2)
  z' = h.sum(rows); z = ro(z') + z'; y = cls(z)

Device mapping (layout B: m on partitions, j on free axis):
  - PE per i-pair: K=128 matmul re-streams bb into PSUM (rhs = hT repeated
    twice -> F=512 covers 2 i's), then a K=1 matmul accumulates
    ww[m] (x) W[i,:] on top (W rows packed on partitions {0,32,64,96} so a
    [1,512] rhs row slice is 32-aligned).
  - Evacuation alternates between ScalarE and VectorE, one instruction per
    (i, m-tile): ACT relu(psum + bias[m]) with free-axis accum_out, or DVE
    scalar_tensor_tensor max(psum + bias, 0) with accum_out.  accum_out
    columns land directly in sT[m, i].
"""

import numpy as np

import concourse.bacc as bacc
import concourse.bass as bass
import concourse.tile as tile
from concourse import mybir
from concourse.bass_utils import run_bass_kernel_spmd

F32 = mybir.dt.float32
F16 = mybir.dt.float16
OP = mybir.AluOpType
AF = mybir.ActivationFunctionType

B, N, D, HID, MSG, RO, L, C = 8, 256, 256, 128, 256, 128, 2, 2
NCORES = 8


def _ap(t, ap, offset=0):
    return bass.AP(tensor=t.tensor, offset=t.offset + offset, ap=ap)


def build_nc():
    nc = bacc.Bacc("TRN2")

    xt = nc.dram_tensor("xt", [128, 2 * N], F16, kind="ExternalInput")
    wpack = nc.dram_tensor("wpack", [128, 64 * N], F16, kind="ExternalInput")
    encw1t = nc.dram_tensor("encw1t", [128, 2 * HID], F16, kind="ExternalInput")
    encw2t = nc.dram_tensor("encw2t", [128, HID], F16, kind="ExternalInput")
    encb1 = nc.dram_tensor("encb1", [128, 1], F32, kind="ExternalInput")
    encb2 = nc.dram_tensor("encb2", [128, 1], F32, kind="ExternalInput")
    watc = nc.dram_tensor("watc", [128, L * MSG], F16, kind="ExternalInput")
    wbtc = nc.dram_tensor("wbtc", [128, L * MSG], F16, kind="ExternalInput")
    ww4 = nc.dram_tensor("ww4", [128, L * 2 * 4 * 128], F16, kind="ExternalInput")
    msgb1 = nc.dram_tensor("msgb1", [128, L * 2], F32, kind="ExternalInput")
    w2tc = nc.dram_tensor("w2tc", [128, L * MSG], F16, kind="ExternalInput")
    nb2 = nc.dram_tensor("nb2", [128, L], F32, kind="ExternalInput")
    row1t = nc.dram_tensor("row1t", [128, RO], F32, kind="ExternalInput")
    rob1 = nc.dram_tensor("rob1", [128, 1], F32, kind="ExternalInput")
    row2t = nc.dram_tensor("row2t", [128, RO], F32, kind="ExternalInput")
    rob2 = nc.dram_tensor("rob2", [128, 1], F32, kind="ExternalInput")
    clswt = nc.dram_tensor("clswt", [128, C], F32, kind="ExternalInput")
    clsb = nc.dram_tensor("clsb", [C, 1], F32, kind="ExternalInput")
    y = nc.dram_tensor("y", [C, 1], F32, kind="ExternalOutput")

    with tile.TileContext(nc) as tc:
        with (
            tc.tile_pool(name="consts", bufs=1) as consts,
            tc.tile_pool(name="work", bufs=2) as work,
            tc.tile_pool(name="mmp", bufs=2, space="PSUM") as mmp,
            tc.tile_pool(name="inner", bufs=6, space="PSUM") as innerp,
        ):
            dma = nc.sync.dma_start

            # ---- constant loads -------------------------------------------
            t_xt = consts.tile([128, 2 * N], F16)
            dma(out=t_xt[:, :], in_=xt[:, :])
            t_encw1t = consts.tile([128, 2 * HID], F16)
            dma(out=t_encw1t[:, :], in_=encw1t[:, :])
            t_encw2t = consts.tile([128, HID], F16)
            dma(out=t_encw2t[:, :], in_=encw2t[:, :])
            t_encb1 = consts.tile([128, 1], F32)
            dma(out=t_encb1[:, :], in_=encb1[:, :])
            t_encb2 = consts.tile([128, 1], F32)
            dma(out=t_encb2[:, :], in_=encb2[:, :])
            t_wat = consts.tile([128, L * MSG], F16)
            dma(out=t_wat[:, :], in_=watc[:, :])
            t_wbt = consts.tile([128, L * MSG], F16)
            dma(out=t_wbt[:, :], in_=wbtc[:, :])
            t_msgb1 = consts.tile([128, L * 2], F32)
            dma(out=t_msgb1[:, :], in_=msgb1[:, :])
            t_w2t = consts.tile([128, L * MSG], F16)
            dma(out=t_w2t[:, :], in_=w2tc[:, :])
            t_nb2 = consts.tile([128, L], F32)
            dma(out=t_nb2[:, :], in_=nb2[:, :])
            t_row1t = consts.tile([128, RO], F32)
            dma(out=t_row1t[:, :], in_=row1t[:, :])
            t_rob1 = consts.tile([128, 1], F32)
            dma(out=t_rob1[:, :], in_=rob1[:, :])
            t_row2t = consts.tile([128, RO], F32)
            dma(out=t_row2t[:, :], in_=row2t[:, :])
            t_rob2 = consts.tile([128, 1], F32)
            dma(out=t_rob2[:, :], in_=rob2[:, :])
            t_clswt = consts.tile([128, C], F32)
            dma(out=t_clswt[:, :], in_=clswt[:, :])
            t_clsb = consts.tile([C, 1], F32)
            dma(out=t_clsb[:, :], in_=clsb[:, :])


            # W packed dense: 8 column chunks, alternating HWDGE/SWDGE
            # queues so transfers overlap; first pairs need only chunk 0.
            t_wpack = consts.tile([128, 64 * N], F16)
            t_ww4 = consts.tile([128, L * 2 * 4 * 128], F16)
            dma(out=t_ww4[:, :], in_=ww4[:, :])
            CH = 64 * N // 8
            for c in range(8):
                eng = dma if c % 2 == 0 else nc.gpsimd.dma_start
                eng(out=t_wpack[:, c * CH:(c + 1) * CH],
                    in_=wpack[:, c * CH:(c + 1) * CH])

            # zero tile for the DVE max() path
            t_zero = consts.tile([128, N], F32)
            nc.vector.memset(t_zero[:, :], 0.0)

            # ---- encoder ---------------------------------------------------
            pe1 = mmp.tile([128, N], F32, tag="mm", name="pe1")
            nc.tensor.matmul(pe1[:, :], t_encw1t[:, 0:128], t_xt[:, 0:N],
                             start=True, stop=False)
            nc.tensor.matmul(pe1[:, :], t_encw1t[:, 128:256], t_xt[:, N:2 * N],
                             start=False, stop=True)
            t_h1 = work.tile([128, N], F16, tag="h1")
            nc.scalar.activation(out=t_h1[:, :], in_=pe1[:, :], func=AF.Relu,
                                 bias=t_encb1[:, 0:1])
            pe2 = mmp.tile([128, N], F32, tag="mm", name="pe2")
            nc.tensor.matmul(pe2[:, :], t_encw2t[:, :], t_h1[:, :],
                             start=True, stop=True)
            t_h = work.tile([128, N], F32, tag="hT")
            nc.vector.tensor_scalar(out=t_h[:, :], in0=pe2[:, :],
                                    scalar1=t_encb2[:, 0:1], scalar2=None,
                                    op0=OP.add)
            t_h16 = work.tile([128, N], F16, tag="hT16")
            nc.vector.tensor_copy(t_h16[:, :], t_h[:, :])

            # ---- message-passing layers -----------------------------------
            for l in range(L):
                # aT (+ msg_b1) per m-tile: bias columns for the inner loop
                t_ab = []
                for mt in range(2):
                    pa = mmp.tile([128, N], F32, tag="mm", name="pa")
                    nc.tensor.matmul(
                        pa[:, :],
                        t_wat[:, l * MSG + mt * 128: l * MSG + (mt + 1) * 128],
                        t_h16[:, :], start=True, stop=True)
                    ab = work.tile([128, N], F32, tag=f"ab{mt}")
                    nc.vector.tensor_scalar(
                        out=ab[:, :], in0=pa[:, :],
                        scalar1=t_msgb1[:, (l * 2 + mt):(l * 2 + mt) + 1],
                        scalar2=None, op0=OP.add)
                    nab = work.tile([128, N], F32, tag=f"nab{mt}", name=f"nab{mt}")
                    nc.vector.tensor_scalar(
                        out=nab[:, :], in0=pa[:, :],
                        scalar1=t_msgb1[:, (l * 2 + mt):(l * 2 + mt) + 1],
                        scalar2=-1.0, op0=OP.add, op1=OP.mult)
                    t_ab.append((ab, nab))

                t_sp = work.tile([128, 2 * N], F32, tag="sTp", name="sTp")
                t_s = [t_sp[:, mt * N:(mt + 1) * N] for mt in range(2)]

                # hT repeated twice along free axis (F=512 -> two i's)
                h_ap = t_h16[:, :]
                h_rep = _ap(h_ap, [h_ap.ap[0], [0, 2], h_ap.ap[1]])

                for mt in range(2):
                    wb_sl = t_wbt[:, l * MSG + mt * 128: l * MSG + (mt + 1) * 128]
                    for p in range(N // 2):
                        i0 = 2 * p
                        G = i0 // 64
                        blk = ((l * 2 + mt) * 4 + G) * 128
                        ofs = (i0 % 64) * N
                        ps = innerp.tile([128, 2 * N], F32)
                        nc.tensor.matmul(ps[:, :], wb_sl, h_rep,
                                         start=True, stop=False)
                        nc.tensor.matmul(
                            ps[:, :],
                            t_ww4[:, blk:blk + 128],
                            t_wpack[:, ofs:ofs + 2 * N],
                            start=False, stop=True)
                        for half in range(2):
                            i = i0 + half
                            sl = ps[:, half * N:(half + 1) * N]
                            if p % 32 < 15:
                                nc.scalar.activation(
                                    out=sl, in_=sl, func=AF.Relu,
                                    bias=t_ab[mt][0][:, i:i + 1],
                                    accum_out=t_sp[:, mt * N + i:mt * N + i + 1])
                            else:
                                # accum = sum_j max(pre-c, -c) = s - N*c;
                                # fixed after the loop on the DVE columns
                                nc.vector.tensor_scalar(
                                    out=sl, in0=sl,
                                    scalar1=t_ab[mt][1][:, i:i + 1],
                                    scalar2=None, op0=OP.max, op1=OP.add,
                                    accum_out=t_sp[:, mt * N + i:mt * N + i + 1])

                # message MLP second linear + relu + residual
                t_s16 = [work.tile([128, N], F16, tag=f"sT16_{mt}", name=f"sT16_{mt}")
                         for mt in range(2)]
                for mt in range(2):
                    base = t_sp[:, mt * N:(mt + 1) * N]
                    dcols = _ap(base, [base.ap[0], [64, N // 64], [1, 34]],
                                offset=30)
                    nabd = _ap(t_ab[mt][1][:, :],
                               [t_ab[mt][1][:, :].ap[0], [64, N // 64], [1, 34]],
                               offset=30)
                    nc.vector.scalar_tensor_tensor(
                        out=dcols, in0=nabd, scalar=-float(N), in1=dcols,
                        op0=OP.mult, op1=OP.add)
                    nc.vector.tensor_copy(t_s16[mt][:, :],
                                          t_sp[:, mt * N:(mt + 1) * N])
                pm = mmp.tile([128, N], F32, tag="mm", name="pm")
                nc.tensor.matmul(pm[:, :], t_w2t[:, (l * 2) * 128:(l * 2 + 1) * 128],
                                 t_s16[0][:, :], start=True, stop=False)
                nc.tensor.matmul(pm[:, :], t_w2t[:, (l * 2 + 1) * 128:(l * 2 + 2) * 128],
                                 t_s16[1][:, :], start=False, stop=True)
                t_md = work.tile([128, N], F32, tag="md")
                nc.scalar.activation(out=t_md[:, :], in_=pm[:, :], func=AF.Relu,
                                     bias=t_nb2[:, l:l + 1])
                t_hn = work.tile([128, N], F32, tag="hT")
                nc.vector.tensor_tensor(out=t_hn[:, :], in0=t_h[:, :],
                                        in1=t_md[:, :], op=OP.add)
                t_h = t_hn
                if l + 1 < L:
                    t_h16 = work.tile([128, N], F16, tag="hT16", name="h16n")
                    nc.vector.tensor_copy(t_h16[:, :], t_h[:, :])

            # ---- readout + classifier -------------------------------------
            t_scr = work.tile([128, N], F32, tag="scr")
            t_z = work.tile([128, 1], F32, tag="zcol")
            nc.scalar.activation(out=t_scr[:, :], in_=t_h[:, :], func=AF.Copy,
                                 accum_out=t_z[:, 0:1])
            pr1 = mmp.tile([128, 1], F32, tag="mm", name="pr1")
            nc.tensor.matmul(pr1[:, :], t_row1t[:, :], t_z[:, 0:1],
                             start=True, stop=True)
            t_r1 = work.tile([128, 1], F32, tag="r1")
            nc.scalar.activation(out=t_r1[:, :], in_=pr1[:, :], func=AF.Relu,
                                 bias=t_rob1[:, 0:1])
            pr2 = mmp.tile([128, 1], F32, tag="mm", name="pr2")
            nc.tensor.matmul(pr2[:, :], t_row2t[:, :], t_r1[:, 0:1],
                             start=True, stop=True)
            t_zf = work.tile([128, 1], F32, tag="zf")
            nc.vector.scalar_tensor_tensor(out=t_zf[:, :], in0=pr2[:, :],
                                           scalar=t_rob2[:, 0:1],
                                           in1=t_z[:, 0:1],
                                           op0=OP.add, op1=OP.add)
            py = mmp.tile([C, 1], F32, tag="mm", name="py")
            nc.tensor.matmul(py[:, :], t_clswt[:, :], t_zf[:, 0:1],
                             start=True, stop=True)
            t_y = work.tile([C, 1], F32, tag="ysb")
            nc.vector.tensor_scalar(out=t_y[:, :], in0=py[:, :],
                                    scalar1=t_clsb[:, 0:1], scalar2=None,
                                    op0=OP.add)
            dma(out=y[:, :], in_=t_y[:, :])

    nc.finalize()
    return nc


def _prep_shared(params):
    """Host-side layout prep of the (tiny) shared parameter tensors."""
    f = np.float32
    enc_w1, enc_b1, enc_w2, enc_b2 = (params["enc_w1"], params["enc_b1"],
                                      params["enc_w2"], params["enc_b2"])
    msg_w1, msg_b1, msg_w2, msg_b2 = (params["msg_w1"], params["msg_b1"],
                                      params["msg_w2"], params["msg_b2"])
    ro_w1, ro_b1, ro_w2, ro_b2 = (params["ro_w1"], params["ro_b1"],
                                  params["ro_w2"], params["ro_b2"])
    cls_w, cls_b = params["cls_w"], params["cls_b"]

    w1t = np.ascontiguousarray(enc_w1.T)          # [D, HID]
    encw1t = np.concatenate([w1t[:128], w1t[128:]], axis=1)  # [128, 256]
    h = np.float16

    watc = np.zeros((128, L * MSG), h)
    wbtc = np.zeros((128, L * MSG), h)
    ww4 = np.zeros((128, L * 2 * 4 * 128), h)
    msgb1c = np.zeros((128, L * 2), f)
    w2tc = np.zeros((128, L * MSG), h)
    nb2c = np.zeros((128, L), f)
    for l in range(L):
        wa = msg_w1[l][:, :HID]                   # [MSG, HID]
        wb = msg_w1[l][:, HID:2 * HID]
        ww = msg_w1[l][:, -1]                     # [MSG]
        watc[:, l * MSG:(l + 1) * MSG] = wa.T
        wbtc[:, l * MSG:(l + 1) * MSG] = wb.T

        w2t = msg_w2[l].T                         # [MSG, HID]
        for mt in range(2):
            msgb1c[:, l * 2 + mt] = msg_b1[l][mt * 128:(mt + 1) * 128]
            for G in range(4):
                blk = ((l * 2 + mt) * 4 + G) * 128
                ww4[32 * G, blk:blk + 128] = ww[mt * 128:(mt + 1) * 128]
            w2tc[:, (l * 2 + mt) * 128:(l * 2 + mt + 1) * 128] = \
                w2t[mt * 128:(mt + 1) * 128]
        nb2c[:, l] = np.float32(N) * msg_b2[l]

    return {
        "encw1t": np.ascontiguousarray(encw1t, h),
        "encw2t": np.ascontiguousarray(enc_w2.T, h),
        "encb1": np.ascontiguousarray(enc_b1.reshape(128, 1), f),
        "encb2": np.ascontiguousarray(enc_b2.reshape(128, 1), f),
        "watc": watc, "wbtc": wbtc, "ww4": ww4, "msgb1": msgb1c,
        "w2tc": w2tc, "nb2": nb2c,
        "row1t": np.ascontiguousarray(ro_w1.T, f),
        "rob1": np.ascontiguousarray(ro_b1.reshape(RO, 1), f),
        "row2t": np.ascontiguousarray(ro_w2.T, f),
        "rob2": np.ascontiguousarray(ro_b2.reshape(RO, 1), f),
        "clswt": np.ascontiguousarray(cls_w.T, f),
        "clsb": np.ascontiguousarray(cls_b.reshape(C, 1), f),
    }


_NC_CACHE = {}


def _get_nc():
    if "nc" not in _NC_CACHE:
        _NC_CACHE["nc"] = build_nc()
    return _NC_CACHE["nc"]


def run(inputs, trace=False):
    inputs = {k: np.asarray(v, dtype=np.float32) for k, v in inputs.items()}
    shared = _prep_shared(inputs)
    W, X = inputs["W"], inputs["X"]
    in_maps = []
    for b in range(NCORES):
        xt = np.ascontiguousarray(X[b].T)         # [D, N]
        m = dict(shared)
        m["xt"] = np.ascontiguousarray(
            np.concatenate([xt[:128], xt[128:]], axis=1), np.float16)
        base = np.float16(W[b]).reshape(4, 64 * N)
        m["wpack"] = np.ascontiguousarray(np.repeat(base, 32, axis=0))
        in_maps.append(m)
    nc = _get_nc()
    res = run_bass_kernel_spmd(nc, in_maps, core_ids=list(range(NCORES)),
                               trace=trace)
    out = np.stack([res.results[b]["y"].reshape(C) for b in range(NCORES)])
    return np.ascontiguousarray(out, np.float32), res


def kernel(**inputs):
    out, _ = run(inputs)
    return out


# revision 23
# speedup vs baseline: 1.5111x; 1.4395x over previous
"""Trainium2 Bass kernel for nn_BrainNN (GNN message passing).

Math per sample b (data-parallel: one sample per NeuronCore):
  h   = enc(X[b])                                   # [N, HID] (kept transposed [HID, N])
  for l in 0,1:
    s[i,m] = sum_j relu(a[i,m] + b1[m] + bb[j,m] + ww[m]*W[i,j])
    h     += relu(s @ w2.T + N*b2)
  z' = h.sum(rows); z = ro(z') + z'; y = cls(z)

Device mapping (layout B: m on partitions, j on free axis):
  - PE per i-pair: K=128 matmul re-streams bb into PSUM (rhs = hT repeated
    twice -> F=512 covers 2 i's), then a K=1 matmul accumulates
    ww[m] (x) W[i,:] on top (W rows packed on partitions {0,32,64,96} so a
    [1,512] rhs row slice is 32-aligned).
  - Evacuation alternates between ScalarE and VectorE, one instruction per
    (i, m-tile): ACT relu(psum + bias[m]) with free-axis accum_out, or DVE
    scalar_tensor_tensor max(psum + bias, 0) with accum_out.  accum_out
    columns land directly in sT[m, i].
"""

import numpy as np

import concourse.bacc as bacc
import concourse.bass as bass
import concourse.tile as tile
from concourse import mybir
from concourse.bass_utils import run_bass_kernel_spmd

F32 = mybir.dt.float32
F16 = mybir.dt.float16
OP = mybir.AluOpType
AF = mybir.ActivationFunctionType

B, N, D, HID, MSG, RO, L, C = 8, 256, 256, 128, 256, 128, 2, 2
NCORES = 8


def _ap(t, ap, offset=0):
    return bass.AP(tensor=t.tensor, offset=t.offset + offset, ap=ap)


def build_nc():
    nc = bacc.Bacc("TRN2")

    xt = nc.dram_tensor("xt", [128, 2 * N], F16, kind="ExternalInput")
    wpack = nc.dram_tensor("wpack", [128, 64 * N], F16, kind="ExternalInput")
    encw1t = nc.dram_tensor("encw1t", [128, 2 * HID], F16, kind="ExternalInput")
    encw2t = nc.dram_tensor("encw2t", [128, HID], F16, kind="ExternalInput")
    encb1 = nc.dram_tensor("encb1", [128, 1], F32, kind="ExternalInput")
    encb2 = nc.dram_tensor("encb2", [128, 1], F32, kind="ExternalInput")
    watc = nc.dram_tensor("watc", [128, L * MSG], F16, kind="ExternalInput")
    wbtc = nc.dram_tensor("wbtc", [128, L * MSG], F16, kind="ExternalInput")
    ww4 = nc.dram_tensor("ww4", [128, L * 2 * 4 * 128], F16, kind="ExternalInput")
    msgb1 = nc.dram_tensor("msgb1", [128, L * 2], F32, kind="ExternalInput")
    w2tc = nc.dram_tensor("w2tc", [128, L * MSG], F16, kind="ExternalInput")
    nb2 = nc.dram_tensor("nb2", [128, L], F32, kind="ExternalInput")
    row1t = nc.dram_tensor("row1t", [128, RO], F32, kind="ExternalInput")
    rob1 = nc.dram_tensor("rob1", [128, 1], F32, kind="ExternalInput")
    row2t = nc.dram_tensor("row2t", [128, RO], F32, kind="ExternalInput")
    rob2 = nc.dram_tensor("rob2", [128, 1], F32, kind="ExternalInput")
    clswt = nc.dram_tensor("clswt", [128, C], F32, kind="ExternalInput")
    clsb = nc.dram_tensor("clsb", [C, 1], F32, kind="ExternalInput")
    y = nc.dram_tensor("y", [C, 1], F32, kind="ExternalOutput")

    with tile.TileContext(nc) as tc:
        with (
            tc.tile_pool(name="consts", bufs=1) as consts,
            tc.tile_pool(name="work", bufs=2) as work,
            tc.tile_pool(name="mmp", bufs=2, space="PSUM") as mmp,
            tc.tile_pool(name="inner", bufs=6, space="PSUM") as innerp,
        ):
            dma = nc.sync.dma_start

            # ---- constant loads -------------------------------------------
            t_xt = consts.tile([128, 2 * N], F16)
            dma(out=t_xt[:, :], in_=xt[:, :])
            t_encw1t = consts.tile([128, 2 * HID], F16)
            dma(out=t_encw1t[:, :], in_=encw1t[:, :])
            t_encw2t = consts.tile([128, HID], F16)
            dma(out=t_encw2t[:, :], in_=encw2t[:, :])
            t_encb1 = consts.tile([128, 1], F32)
            dma(out=t_encb1[:, :], in_=encb1[:, :])
            t_encb2 = consts.tile([128, 1], F32)
            dma(out=t_encb2[:, :], in_=encb2[:, :])
            t_wat = consts.tile([128, L * MSG], F16)
            dma(out=t_wat[:, :], in_=watc[:, :])
            t_wbt = consts.tile([128, L * MSG], F16)
            dma(out=t_wbt[:, :], in_=wbtc[:, :])
            t_msgb1 = consts.tile([128, L * 2], F32)
            dma(out=t_msgb1[:, :], in_=msgb1[:, :])
            t_w2t = consts.tile([128, L * MSG], F16)
            dma(out=t_w2t[:, :], in_=w2tc[:, :])
            t_nb2 = consts.tile([128, L], F32)
            dma(out=t_nb2[:, :], in_=nb2[:, :])
            t_row1t = consts.tile([128, RO], F32)
            dma(out=t_row1t[:, :], in_=row1t[:, :])
            t_rob1 = consts.tile([128, 1], F32)
            dma(out=t_rob1[:, :], in_=rob1[:, :])
            t_row2t = consts.tile([128, RO], F32)
            dma(out=t_row2t[:, :], in_=row2t[:, :])
            t_rob2 = consts.tile([128, 1], F32)
            dma(out=t_rob2[:, :], in_=rob2[:, :])
            t_clswt = consts.tile([128, C], F32)
            dma(out=t_clswt[:, :], in_=clswt[:, :])
            t_clsb = consts.tile([C, 1], F32)
            dma(out=t_clsb[:, :], in_=clsb[:, :])


            # W packed dense: 8 column chunks, alternating HWDGE/SWDGE
            # queues so transfers overlap; first pairs need only chunk 0.
            t_wpack = consts.tile([128, 64 * N], F16)
            t_ww4 = consts.tile([128, L * 2 * 4 * 128], F16)
            dma(out=t_ww4[:, :], in_=ww4[:, :])
            CH = 64 * N // 8
            for c in range(8):
                eng = dma if c % 2 == 0 else nc.gpsimd.dma_start
                eng(out=t_wpack[:, c * CH:(c + 1) * CH],
                    in_=wpack[:, c * CH:(c + 1) * CH])

            # zero tile for the DVE max() path
            t_zero = consts.tile([128, N], F32)
            nc.vector.memset(t_zero[:, :], 0.0)

            # ---- encoder ---------------------------------------------------
            pe1 = mmp.tile([128, N], F32, tag="mm", name="pe1")
            nc.tensor.matmul(pe1[:, :], t_encw1t[:, 0:128], t_xt[:, 0:N],
                             start=True, stop=False)
            nc.tensor.matmul(pe1[:, :], t_encw1t[:, 128:256], t_xt[:, N:2 * N],
                             start=False, stop=True)
            t_h1 = work.tile([128, N], F16, tag="h1")
            nc.scalar.activation(out=t_h1[:, :], in_=pe1[:, :], func=AF.Relu,
                                 bias=t_encb1[:, 0:1])
            pe2 = mmp.tile([128, N], F32, tag="mm", name="pe2")
            nc.tensor.matmul(pe2[:, :], t_encw2t[:, :], t_h1[:, :],
                             start=True, stop=True)
            t_h = work.tile([128, N], F32, tag="hT")
            nc.vector.tensor_scalar(out=t_h[:, :], in0=pe2[:, :],
                                    scalar1=t_encb2[:, 0:1], scalar2=None,
                                    op0=OP.add)
            t_h16 = work.tile([128, N], F16, tag="hT16")
            nc.vector.tensor_copy(t_h16[:, :], t_h[:, :])

            # ---- message-passing layers -----------------------------------
            for l in range(L):
                # aT (+ msg_b1) per m-tile: bias columns for the inner loop
                t_ab = []
                for mt in range(2):
                    pa = mmp.tile([128, N], F32, tag="mm", name="pa")
                    nc.tensor.matmul(
                        pa[:, :],
                        t_wat[:, l * MSG + mt * 128: l * MSG + (mt + 1) * 128],
                        t_h16[:, :], start=True, stop=True)
                    ab = work.tile([128, N], F32, tag=f"ab{mt}")
                    nc.vector.tensor_scalar(
                        out=ab[:, :], in0=pa[:, :],
                        scalar1=t_msgb1[:, (l * 2 + mt):(l * 2 + mt) + 1],
                        scalar2=None, op0=OP.add)
                    nab = work.tile([128, N], F32, tag=f"nab{mt}", name=f"nab{mt}")
                    nc.vector.tensor_scalar(
                        out=nab[:, :], in0=pa[:, :],
                        scalar1=t_msgb1[:, (l * 2 + mt):(l * 2 + mt) + 1],
                        scalar2=-1.0, op0=OP.add, op1=OP.mult)
                    t_ab.append((ab, nab))

                t_sp = work.tile([128, 2 * N], F32, tag="sTp", name="sTp")
                t_s = [t_sp[:, mt * N:(mt + 1) * N] for mt in range(2)]

                # hT repeated twice along free axis (F=512 -> two i's)
                h_ap = t_h16[:, :]
                h_rep = _ap(h_ap, [h_ap.ap[0], [0, 2], h_ap.ap[1]])

                for mt in range(2):
                    wb_sl = t_wbt[:, l * MSG + mt * 128: l * MSG + (mt + 1) * 128]
                    for p in range(N // 2):
                        i0 = 2 * p
                        G = i0 // 64
                        blk = ((l * 2 + mt) * 4 + G) * 128
                        ofs = (i0 % 64) * N
                        ps = innerp.tile([128, 2 * N], F32)
                        nc.tensor.matmul(ps[:, :], wb_sl, h_rep,
                                         start=True, stop=False)
                        nc.tensor.matmul(
                            ps[:, :],
                            t_ww4[:, blk:blk + 128],
                            t_wpack[:, ofs:ofs + 2 * N],
                            start=False, stop=True)
                        for half in range(2):
                            i = i0 + half
                            sl = ps[:, half * N:(half + 1) * N]
                            if p % 2 == 0 and p % 16 != 0:
                                nc.scalar.activation(
                                    out=sl, in_=sl, func=AF.Relu,
                                    bias=t_ab[mt][0][:, i:i + 1],
                                    accum_out=t_sp[:, mt * N + i:mt * N + i + 1])
                            else:
                                # accum = sum_j max(pre-c, -c) = s - N*c;
                                # fixed after the loop on the DVE columns
                                nc.vector.tensor_scalar(
                                    out=sl, in0=sl,
                                    scalar1=t_ab[mt][1][:, i:i + 1],
                                    scalar2=None, op0=OP.max, op1=OP.add,
                                    accum_out=t_sp[:, mt * N + i:mt * N + i + 1])

                # message MLP second linear + relu + residual
                t_s16 = [work.tile([128, N], F16, tag=f"sT16_{mt}", name=f"sT16_{mt}")
                         for mt in range(2)]
                for mt in range(2):
                    base = t_sp[:, mt * N:(mt + 1) * N]
                    nabt = t_ab[mt][1][:, :]
                    # DVE pairs: p==0 (mod 16) -> cols 0..3 per 32; p odd ->
                    # cols 4j+2,4j+3 for j=1..7 per 32
                    for fpat, off in (([[4, N // 4], [1, 2]], 2),
                                      ([[32, N // 32], [1, 2]], 0)):
                        dcols = _ap(base, [base.ap[0]] + fpat, offset=off)
                        nabd = _ap(nabt, [nabt.ap[0]] + fpat, offset=off)
                        nc.vector.scalar_tensor_tensor(
                            out=dcols, in0=nabd, scalar=-float(N), in1=dcols,
                            op0=OP.mult, op1=OP.add)
                    nc.vector.tensor_copy(t_s16[mt][:, :],
                                          t_sp[:, mt * N:(mt + 1) * N])
                pm = mmp.tile([128, N], F32, tag="mm", name="pm")
                nc.tensor.matmul(pm[:, :], t_w2t[:, (l * 2) * 128:(l * 2 + 1) * 128],
                                 t_s16[0][:, :], start=True, stop=False)
                nc.tensor.matmul(pm[:, :], t_w2t[:, (l * 2 + 1) * 128:(l * 2 + 2) * 128],
                                 t_s16[1][:, :], start=False, stop=True)
                t_md = work.tile([128, N], F32, tag="md")
                nc.scalar.activation(out=t_md[:, :], in_=pm[:, :], func=AF.Relu,
                                     bias=t_nb2[:, l:l + 1])
                t_hn = work.tile([128, N], F32, tag="hT")
                nc.vector.tensor_tensor(out=t_hn[:, :], in0=t_h[:, :],
                                        in1=t_md[:, :], op=OP.add)
                t_h = t_hn
                if l + 1 < L:
                    t_h16 = work.tile([128, N], F16, tag="hT16", name="h16n")
                    nc.vector.tensor_copy(t_h16[:, :], t_h[:, :])

            # ---- readout + classifier -------------------------------------
            t_scr = work.tile([128, N], F32, tag="scr")
            t_z = work.tile([128, 1], F32, tag="zcol")
            nc.scalar.activation(out=t_scr[:, :], in_=t_h[:, :], func=AF.Copy,
                                 accum_out=t_z[:, 0:1])
            pr1 = mmp.tile([128, 1], F32, tag="mm", name="pr1")
            nc.tensor.matmul(pr1[:, :], t_row1t[:, :], t_z[:, 0:1],
                             start=True, stop=True)
            t_r1 = work.tile([128, 1], F32, tag="r1")
            nc.scalar.activation(out=t_r1[:, :], in_=pr1[:, :], func=AF.Relu,
                                 bias=t_rob1[:, 0:1])
            pr2 = mmp.tile([128, 1], F32, tag="mm", name="pr2")
            nc.tensor.matmul(pr2[:, :], t_row2t[:, :], t_r1[:, 0:1],
                             start=True, stop=True)
            t_zf = work.tile([128, 1], F32, tag="zf")
            nc.vector.scalar_tensor_tensor(out=t_zf[:, :], in0=pr2[:, :],
                                           scalar=t_rob2[:, 0:1],
                                           in1=t_z[:, 0:1],
                                           op0=OP.add, op1=OP.add)
            py = mmp.tile([C, 1], F32, tag="mm", name="py")
            nc.tensor.matmul(py[:, :], t_clswt[:, :], t_zf[:, 0:1],
                             start=True, stop=True)
            t_y = work.tile([C, 1], F32, tag="ysb")
            nc.vector.tensor_scalar(out=t_y[:, :], in0=py[:, :],
                                    scalar1=t_clsb[:, 0:1], scalar2=None,
                                    op0=OP.add)
            dma(out=y[:, :], in_=t_y[:, :])

    nc.finalize()
    return nc


def _prep_shared(params):
    """Host-side layout prep of the (tiny) shared parameter tensors."""
    f = np.float32
    enc_w1, enc_b1, enc_w2, enc_b2 = (params["enc_w1"], params["enc_b1"],
                                      params["enc_w2"], params["enc_b2"])
    msg_w1, msg_b1, msg_w2, msg_b2 = (params["msg_w1"], params["msg_b1"],
                                      params["msg_w2"], params["msg_b2"])
    ro_w1, ro_b1, ro_w2, ro_b2 = (params["ro_w1"], params["ro_b1"],
                                  params["ro_w2"], params["ro_b2"])
    cls_w, cls_b = params["cls_w"], params["cls_b"]

    w1t = np.ascontiguousarray(enc_w1.T)          # [D, HID]
    encw1t = np.concatenate([w1t[:128], w1t[128:]], axis=1)  # [128, 256]
    h = np.float16

    watc = np.zeros((128, L * MSG), h)
    wbtc = np.zeros((128, L * MSG), h)
    ww4 = np.zeros((128, L * 2 * 4 * 128), h)
    msgb1c = np.zeros((128, L * 2), f)
    w2tc = np.zeros((128, L * MSG), h)
    nb2c = np.zeros((128, L), f)
    for l in range(L):
        wa = msg_w1[l][:, :HID]                   # [MSG, HID]
        wb = msg_w1[l][:, HID:2 * HID]
        ww = msg_w1[l][:, -1]                     # [MSG]
        watc[:, l * MSG:(l + 1) * MSG] = wa.T
        wbtc[:, l * MSG:(l + 1) * MSG] = wb.T

        w2t = msg_w2[l].T                         # [MSG, HID]
        for mt in range(2):
            msgb1c[:, l * 2 + mt] = msg_b1[l][mt * 128:(mt + 1) * 128]
            for G in range(4):
                blk = ((l * 2 + mt) * 4 + G) * 128
                ww4[32 * G, blk:blk + 128] = ww[mt * 128:(mt + 1) * 128]
            w2tc[:, (l * 2 + mt) * 128:(l * 2 + mt + 1) * 128] = \
                w2t[mt * 128:(mt + 1) * 128]
        nb2c[:, l] = np.float32(N) * msg_b2[l]

    return {
        "encw1t": np.ascontiguousarray(encw1t, h),
        "encw2t": np.ascontiguousarray(enc_w2.T, h),
        "encb1": np.ascontiguousarray(enc_b1.reshape(128, 1), f),
        "encb2": np.ascontiguousarray(enc_b2.reshape(128, 1), f),
        "watc": watc, "wbtc": wbtc, "ww4": ww4, "msgb1": msgb1c,
        "w2tc": w2tc, "nb2": nb2c,
        "row1t": np.ascontiguousarray(ro_w1.T, f),
        "rob1": np.ascontiguousarray(ro_b1.reshape(RO, 1), f),
        "row2t": np.ascontiguousarray(ro_w2.T, f),
        "rob2": np.ascontiguousarray(ro_b2.reshape(RO, 1), f),
        "clswt": np.ascontiguousarray(cls_w.T, f),
        "clsb": np.ascontiguousarray(cls_b.reshape(C, 1), f),
    }


_NC_CACHE = {}


def _get_nc():
    if "nc" not in _NC_CACHE:
        _NC_CACHE["nc"] = build_nc()
    return _NC_CACHE["nc"]


def run(inputs, trace=False):
    inputs = {k: np.asarray(v, dtype=np.float32) for k, v in inputs.items()}
    shared = _prep_shared(inputs)
    W, X = inputs["W"], inputs["X"]
    in_maps = []
    for b in range(NCORES):
        xt = np.ascontiguousarray(X[b].T)         # [D, N]
        m = dict(shared)
        m["xt"] = np.ascontiguousarray(
            np.concatenate([xt[:128], xt[128:]], axis=1), np.float16)
        base = np.float16(W[b]).reshape(4, 64 * N)
        m["wpack"] = np.ascontiguousarray(np.repeat(base, 32, axis=0))
        in_maps.append(m)
    nc = _get_nc()
    res = run_bass_kernel_spmd(nc, in_maps, core_ids=list(range(NCORES)),
                               trace=trace)
    out = np.stack([res.results[b]["y"].reshape(C) for b in range(NCORES)])
    return np.ascontiguousarray(out, np.float32), res


def kernel(**inputs):
    out, _ = run(inputs)
    return out


# revision 24
# speedup vs baseline: 1.5277x; 1.0110x over previous
"""Trainium2 Bass kernel for nn_BrainNN (GNN message passing).

Math per sample b (data-parallel: one sample per NeuronCore):
  h   = enc(X[b])                                   # [N, HID] (kept transposed [HID, N])
  for l in 0,1:
    s[i,m] = sum_j relu(a[i,m] + b1[m] + bb[j,m] + ww[m]*W[i,j])
    h     += relu(s @ w2.T + N*b2)
  z' = h.sum(rows); z = ro(z') + z'; y = cls(z)

Device mapping (layout B: m on partitions, j on free axis):
  - PE per i-pair: K=128 matmul re-streams bb into PSUM (rhs = hT repeated
    twice -> F=512 covers 2 i's), then a K=1 matmul accumulates
    ww[m] (x) W[i,:] on top (W rows packed on partitions {0,32,64,96} so a
    [1,512] rhs row slice is 32-aligned).
  - Evacuation alternates between ScalarE and VectorE, one instruction per
    (i, m-tile): ACT relu(psum + bias[m]) with free-axis accum_out, or DVE
    scalar_tensor_tensor max(psum + bias, 0) with accum_out.  accum_out
    columns land directly in sT[m, i].
"""

import numpy as np

import concourse.bacc as bacc
import concourse.bass as bass
import concourse.tile as tile
from concourse import mybir
from concourse.bass_utils import run_bass_kernel_spmd

F32 = mybir.dt.float32
F16 = mybir.dt.float16
OP = mybir.AluOpType
AF = mybir.ActivationFunctionType

B, N, D, HID, MSG, RO, L, C = 8, 256, 256, 128, 256, 128, 2, 2
NCORES = 8


def _ap(t, ap, offset=0):
    return bass.AP(tensor=t.tensor, offset=t.offset + offset, ap=ap)


def build_nc():
    nc = bacc.Bacc("TRN2")

    xt = nc.dram_tensor("xt", [128, 2 * N], F16, kind="ExternalInput")
    wpack = nc.dram_tensor("wpack", [128, 64 * N], F16, kind="ExternalInput")
    encw1t = nc.dram_tensor("encw1t", [128, 2 * HID], F16, kind="ExternalInput")
    encw2t = nc.dram_tensor("encw2t", [128, HID], F16, kind="ExternalInput")
    encb1 = nc.dram_tensor("encb1", [128, 1], F32, kind="ExternalInput")
    encb2 = nc.dram_tensor("encb2", [128, 1], F32, kind="ExternalInput")
    watc = nc.dram_tensor("watc", [128, L * MSG], F16, kind="ExternalInput")
    wbtc = nc.dram_tensor("wbtc", [128, L * MSG], F16, kind="ExternalInput")
    ww4 = nc.dram_tensor("ww4", [128, L * 2 * 4 * 128], F16, kind="ExternalInput")
    msgb1 = nc.dram_tensor("msgb1", [128, L * 2], F32, kind="ExternalInput")
    w2tc = nc.dram_tensor("w2tc", [128, L * MSG], F16, kind="ExternalInput")
    nb2 = nc.dram_tensor("nb2", [128, L], F32, kind="ExternalInput")
    row1t = nc.dram_tensor("row1t", [128, RO], F32, kind="ExternalInput")
    rob1 = nc.dram_tensor("rob1", [128, 1], F32, kind="ExternalInput")
    row2t = nc.dram_tensor("row2t", [128, RO], F32, kind="ExternalInput")
    rob2 = nc.dram_tensor("rob2", [128, 1], F32, kind="ExternalInput")
    clswt = nc.dram_tensor("clswt", [128, C], F32, kind="ExternalInput")
    clsb = nc.dram_tensor("clsb", [C, 1], F32, kind="ExternalInput")
    y = nc.dram_tensor("y", [C, 1], F32, kind="ExternalOutput")

    with tile.TileContext(nc) as tc:
        with (
            tc.tile_pool(name="consts", bufs=1) as consts,
            tc.tile_pool(name="work", bufs=2) as work,
            tc.tile_pool(name="mmp", bufs=2, space="PSUM") as mmp,
            tc.tile_pool(name="inner", bufs=6, space="PSUM") as innerp,
        ):
            dma = nc.sync.dma_start

            # ---- constant loads -------------------------------------------
            t_xt = consts.tile([128, 2 * N], F16)
            dma(out=t_xt[:, :], in_=xt[:, :])
            t_encw1t = consts.tile([128, 2 * HID], F16)
            dma(out=t_encw1t[:, :], in_=encw1t[:, :])
            t_encw2t = consts.tile([128, HID], F16)
            dma(out=t_encw2t[:, :], in_=encw2t[:, :])
            t_encb1 = consts.tile([128, 1], F32)
            dma(out=t_encb1[:, :], in_=encb1[:, :])
            t_encb2 = consts.tile([128, 1], F32)
            dma(out=t_encb2[:, :], in_=encb2[:, :])
            t_wat = consts.tile([128, L * MSG], F16)
            dma(out=t_wat[:, :], in_=watc[:, :])
            t_wbt = consts.tile([128, L * MSG], F16)
            dma(out=t_wbt[:, :], in_=wbtc[:, :])
            t_msgb1 = consts.tile([128, L * 2], F32)
            dma(out=t_msgb1[:, :], in_=msgb1[:, :])
            t_w2t = consts.tile([128, L * MSG], F16)
            dma(out=t_w2t[:, :], in_=w2tc[:, :])
            t_nb2 = consts.tile([128, L], F32)
            dma(out=t_nb2[:, :], in_=nb2[:, :])
            t_row1t = consts.tile([128, RO], F32)
            dma(out=t_row1t[:, :], in_=row1t[:, :])
            t_rob1 = consts.tile([128, 1], F32)
            dma(out=t_rob1[:, :], in_=rob1[:, :])
            t_row2t = consts.tile([128, RO], F32)
            dma(out=t_row2t[:, :], in_=row2t[:, :])
            t_rob2 = consts.tile([128, 1], F32)
            dma(out=t_rob2[:, :], in_=rob2[:, :])
            t_clswt = consts.tile([128, C], F32)
            dma(out=t_clswt[:, :], in_=clswt[:, :])
            t_clsb = consts.tile([C, 1], F32)
            dma(out=t_clsb[:, :], in_=clsb[:, :])


            # W packed dense: 8 column chunks, alternating HWDGE/SWDGE
            # queues so transfers overlap; first pairs need only chunk 0.
            t_wpack = consts.tile([128, 64 * N], F16)
            t_ww4 = consts.tile([128, L * 2 * 4 * 128], F16)
            dma(out=t_ww4[:, :], in_=ww4[:, :])
            CH = 64 * N // 8
            for c in range(8):
                dma(out=t_wpack[:, c * CH:(c + 1) * CH],
                    in_=wpack[:, c * CH:(c + 1) * CH])

            # zero tile for the DVE max() path
            t_zero = consts.tile([128, N], F32)
            nc.vector.memset(t_zero[:, :], 0.0)

            # ---- encoder ---------------------------------------------------
            pe1 = mmp.tile([128, N], F32, tag="mm", name="pe1")
            nc.tensor.matmul(pe1[:, :], t_encw1t[:, 0:128], t_xt[:, 0:N],
                             start=True, stop=False)
            nc.tensor.matmul(pe1[:, :], t_encw1t[:, 128:256], t_xt[:, N:2 * N],
                             start=False, stop=True)
            t_h1 = work.tile([128, N], F16, tag="h1")
            nc.scalar.activation(out=t_h1[:, :], in_=pe1[:, :], func=AF.Relu,
                                 bias=t_encb1[:, 0:1])
            pe2 = mmp.tile([128, N], F32, tag="mm", name="pe2")
            nc.tensor.matmul(pe2[:, :], t_encw2t[:, :], t_h1[:, :],
                             start=True, stop=True)
            t_h = work.tile([128, N], F32, tag="hT")
            nc.vector.tensor_scalar(out=t_h[:, :], in0=pe2[:, :],
                                    scalar1=t_encb2[:, 0:1], scalar2=None,
                                    op0=OP.add)
            t_h16 = work.tile([128, N], F16, tag="hT16")
            nc.vector.tensor_copy(t_h16[:, :], t_h[:, :])

            # ---- message-passing layers -----------------------------------
            for l in range(L):
                # aT (+ msg_b1) per m-tile: bias columns for the inner loop
                t_ab = []
                for mt in range(2):
                    pa = mmp.tile([128, N], F32, tag="mm", name="pa")
                    nc.tensor.matmul(
                        pa[:, :],
                        t_wat[:, l * MSG + mt * 128: l * MSG + (mt + 1) * 128],
                        t_h16[:, :], start=True, stop=True)
                    ab = work.tile([128, N], F32, tag=f"ab{mt}")
                    nc.vector.tensor_scalar(
                        out=ab[:, :], in0=pa[:, :],
                        scalar1=t_msgb1[:, (l * 2 + mt):(l * 2 + mt) + 1],
                        scalar2=None, op0=OP.add)
                    nab = work.tile([128, N], F32, tag=f"nab{mt}", name=f"nab{mt}")
                    nc.vector.tensor_scalar(
                        out=nab[:, :], in0=pa[:, :],
                        scalar1=t_msgb1[:, (l * 2 + mt):(l * 2 + mt) + 1],
                        scalar2=-1.0, op0=OP.add, op1=OP.mult)
                    t_ab.append((ab, nab))

                t_sp = work.tile([128, 2 * N], F32, tag="sTp", name="sTp")
                t_s = [t_sp[:, mt * N:(mt + 1) * N] for mt in range(2)]

                # hT repeated twice along free axis (F=512 -> two i's)
                h_ap = t_h16[:, :]
                h_rep = _ap(h_ap, [h_ap.ap[0], [0, 2], h_ap.ap[1]])

                for mt in range(2):
                    wb_sl = t_wbt[:, l * MSG + mt * 128: l * MSG + (mt + 1) * 128]
                    for p in range(N // 2):
                        i0 = 2 * p
                        G = i0 // 64
                        blk = ((l * 2 + mt) * 4 + G) * 128
                        ofs = (i0 % 64) * N
                        ps = innerp.tile([128, 2 * N], F32)
                        nc.tensor.matmul(ps[:, :], wb_sl, h_rep,
                                         start=True, stop=False)
                        nc.tensor.matmul(
                            ps[:, :],
                            t_ww4[:, blk:blk + 128],
                            t_wpack[:, ofs:ofs + 2 * N],
                            start=False, stop=True)
                        for half in range(2):
                            i = i0 + half
                            sl = ps[:, half * N:(half + 1) * N]
                            if p % 2 == 0 and p % 16 != 0:
                                nc.scalar.activation(
                                    out=sl, in_=sl, func=AF.Relu,
                                    bias=t_ab[mt][0][:, i:i + 1],
                                    accum_out=t_sp[:, mt * N + i:mt * N + i + 1])
                            else:
                                # accum = sum_j max(pre-c, -c) = s - N*c;
                                # fixed after the loop on the DVE columns
                                nc.vector.tensor_scalar(
                                    out=sl, in0=sl,
                                    scalar1=t_ab[mt][1][:, i:i + 1],
                                    scalar2=None, op0=OP.max, op1=OP.add,
                                    accum_out=t_sp[:, mt * N + i:mt * N + i + 1])

                # message MLP second linear + relu + residual
                t_s16 = [work.tile([128, N], F16, tag=f"sT16_{mt}", name=f"sT16_{mt}")
                         for mt in range(2)]
                for mt in range(2):
                    base = t_sp[:, mt * N:(mt + 1) * N]
                    nabt = t_ab[mt][1][:, :]
                    # DVE pairs: p==0 (mod 16) -> cols 0..3 per 32; p odd ->
                    # cols 4j+2,4j+3 for j=1..7 per 32
                    for fpat, off in (([[4, N // 4], [1, 2]], 2),
                                      ([[32, N // 32], [1, 2]], 0)):
                        dcols = _ap(base, [base.ap[0]] + fpat, offset=off)
                        nabd = _ap(nabt, [nabt.ap[0]] + fpat, offset=off)
                        nc.vector.scalar_tensor_tensor(
                            out=dcols, in0=nabd, scalar=-float(N), in1=dcols,
                            op0=OP.mult, op1=OP.add)
                    nc.vector.tensor_copy(t_s16[mt][:, :],
                                          t_sp[:, mt * N:(mt + 1) * N])
                pm = mmp.tile([128, N], F32, tag="mm", name="pm")
                nc.tensor.matmul(pm[:, :], t_w2t[:, (l * 2) * 128:(l * 2 + 1) * 128],
                                 t_s16[0][:, :], start=True, stop=False)
                nc.tensor.matmul(pm[:, :], t_w2t[:, (l * 2 + 1) * 128:(l * 2 + 2) * 128],
                                 t_s16[1][:, :], start=False, stop=True)
                t_md = work.tile([128, N], F32, tag="md")
                nc.scalar.activation(out=t_md[:, :], in_=pm[:, :], func=AF.Relu,
                                     bias=t_nb2[:, l:l + 1])
                t_hn = work.tile([128, N], F32, tag="hT")
                nc.vector.tensor_tensor(out=t_hn[:, :], in0=t_h[:, :],
                                        in1=t_md[:, :], op=OP.add)
                t_h = t_hn
                if l + 1 < L:
                    t_h16 = work.tile([128, N], F16, tag="hT16", name="h16n")
                    nc.vector.tensor_copy(t_h16[:, :], t_h[:, :])

            # ---- readout + classifier -------------------------------------
            t_scr = work.tile([128, N], F32, tag="scr")
            t_z = work.tile([128, 1], F32, tag="zcol")
            nc.scalar.activation(out=t_scr[:, :], in_=t_h[:, :], func=AF.Copy,
                                 accum_out=t_z[:, 0:1])
            pr1 = mmp.tile([128, 1], F32, tag="mm", name="pr1")
            nc.tensor.matmul(pr1[:, :], t_row1t[:, :], t_z[:, 0:1],
                             start=True, stop=True)
            t_r1 = work.tile([128, 1], F32, tag="r1")
            nc.scalar.activation(out=t_r1[:, :], in_=pr1[:, :], func=AF.Relu,
                                 bias=t_rob1[:, 0:1])
            pr2 = mmp.tile([128, 1], F32, tag="mm", name="pr2")
            nc.tensor.matmul(pr2[:, :], t_row2t[:, :], t_r1[:, 0:1],
                             start=True, stop=True)
            t_zf = work.tile([128, 1], F32, tag="zf")
            nc.vector.scalar_tensor_tensor(out=t_zf[:, :], in0=pr2[:, :],
                                           scalar=t_rob2[:, 0:1],
                                           in1=t_z[:, 0:1],
                                           op0=OP.add, op1=OP.add)
            py = mmp.tile([C, 1], F32, tag="mm", name="py")
            nc.tensor.matmul(py[:, :], t_clswt[:, :], t_zf[:, 0:1],
                             start=True, stop=True)
            t_y = work.tile([C, 1], F32, tag="ysb")
            nc.vector.tensor_scalar(out=t_y[:, :], in0=py[:, :],
                                    scalar1=t_clsb[:, 0:1], scalar2=None,
                                    op0=OP.add)
            dma(out=y[:, :], in_=t_y[:, :])

    nc.finalize()
    return nc


def _prep_shared(params):
    """Host-side layout prep of the (tiny) shared parameter tensors."""
    f = np.float32
    enc_w1, enc_b1, enc_w2, enc_b2 = (params["enc_w1"], params["enc_b1"],
                                      params["enc_w2"], params["enc_b2"])
    msg_w1, msg_b1, msg_w2, msg_b2 = (params["msg_w1"], params["msg_b1"],
                                      params["msg_w2"], params["msg_b2"])
    ro_w1, ro_b1, ro_w2, ro_b2 = (params["ro_w1"], params["ro_b1"],
                                  params["ro_w2"], params["ro_b2"])
    cls_w, cls_b = params["cls_w"], params["cls_b"]

    w1t = np.ascontiguousarray(enc_w1.T)          # [D, HID]
    encw1t = np.concatenate([w1t[:128], w1t[128:]], axis=1)  # [128, 256]
    h = np.float16

    watc = np.zeros((128, L * MSG), h)
    wbtc = np.zeros((128, L * MSG), h)
    ww4 = np.zeros((128, L * 2 * 4 * 128), h)
    msgb1c = np.zeros((128, L * 2), f)
    w2tc = np.zeros((128, L * MSG), h)
    nb2c = np.zeros((128, L), f)
    for l in range(L):
        wa = msg_w1[l][:, :HID]                   # [MSG, HID]
        wb = msg_w1[l][:, HID:2 * HID]
        ww = msg_w1[l][:, -1]                     # [MSG]
        watc[:, l * MSG:(l + 1) * MSG] = wa.T
        wbtc[:, l * MSG:(l + 1) * MSG] = wb.T

        w2t = msg_w2[l].T                         # [MSG, HID]
        for mt in range(2):
            msgb1c[:, l * 2 + mt] = msg_b1[l][mt * 128:(mt + 1) * 128]
            for G in range(4):
                blk = ((l * 2 + mt) * 4 + G) * 128
                ww4[32 * G, blk:blk + 128] = ww[mt * 128:(mt + 1) * 128]
            w2tc[:, (l * 2 + mt) * 128:(l * 2 + mt + 1) * 128] = \
                w2t[mt * 128:(mt + 1) * 128]
        nb2c[:, l] = np.float32(N) * msg_b2[l]

    return {
        "encw1t": np.ascontiguousarray(encw1t, h),
        "encw2t": np.ascontiguousarray(enc_w2.T, h),
        "encb1": np.ascontiguousarray(enc_b1.reshape(128, 1), f),
        "encb2": np.ascontiguousarray(enc_b2.reshape(128, 1), f),
        "watc": watc, "wbtc": wbtc, "ww4": ww4, "msgb1": msgb1c,
        "w2tc": w2tc, "nb2": nb2c,
        "row1t": np.ascontiguousarray(ro_w1.T, f),
        "rob1": np.ascontiguousarray(ro_b1.reshape(RO, 1), f),
        "row2t": np.ascontiguousarray(ro_w2.T, f),
        "rob2": np.ascontiguousarray(ro_b2.reshape(RO, 1), f),
        "clswt": np.ascontiguousarray(cls_w.T, f),
        "clsb": np.ascontiguousarray(cls_b.reshape(C, 1), f),
    }


_NC_CACHE = {}


def _get_nc():
    if "nc" not in _NC_CACHE:
        _NC_CACHE["nc"] = build_nc()
    return _NC_CACHE["nc"]


def run(inputs, trace=False):
    inputs = {k: np.asarray(v, dtype=np.float32) for k, v in inputs.items()}
    shared = _prep_shared(inputs)
    W, X = inputs["W"], inputs["X"]
    in_maps = []
    for b in range(NCORES):
        xt = np.ascontiguousarray(X[b].T)         # [D, N]
        m = dict(shared)
        m["xt"] = np.ascontiguousarray(
            np.concatenate([xt[:128], xt[128:]], axis=1), np.float16)
        base = np.float16(W[b]).reshape(4, 64 * N)
        m["wpack"] = np.ascontiguousarray(np.repeat(base, 32, axis=0))
        in_maps.append(m)
    nc = _get_nc()
    res = run_bass_kernel_spmd(nc, in_maps, core_ids=list(range(NCORES)),
                               trace=trace)
    out = np.stack([res.results[b]["y"].reshape(C) for b in range(NCORES)])
    return np.ascontiguousarray(out, np.float32), res


def kernel(**inputs):
    out, _ = run(inputs)
    return out


# revision 25
# speedup vs baseline: 1.5292x; 1.0010x over previous
"""Trainium2 Bass kernel for nn_BrainNN (GNN message passing).

Math per sample b (data-parallel: one sample per NeuronCore):
  h   = enc(X[b])                                   # [N, HID] (kept transposed [HID, N])
  for l in 0,1:
    s[i,m] = sum_j relu(a[i,m] + b1[m] + bb[j,m] + ww[m]*W[i,j])
    h     += relu(s @ w2.T + N*b2)
  z' = h.sum(rows); z = ro(z') + z'; y = cls(z)

Device mapping (layout B: m on partitions, j on free axis), all hot-path
matmuls in fp16 (PSUM accumulates fp32; ~2e-4 final rel err):
  - PE per i-pair (i0, i0+1): one K=128 matmul re-streams bb into a PSUM bank
    (rhs = hT repeated twice -> F=512 covers both i's), then a second K=128
    matmul accumulates ww[m] (x) W[i,:] on top.  The second matmul uses a
    one-hot lhsT (ww on partition 32G, zeros elsewhere) against a dense W
    pack whose 32-partition groups replicate W row-blocks: small-K matmuls
    (K<=32) never register as busy in the PE HAM activity monitor and pin
    the PE at 1.2 GHz; K=128 keeps it at 2.4 GHz.
  - Evacuation alternates ScalarE/VectorE per PAIR (fine interleave, runs
    <= 2, ACT fraction 7/16 matching measured per-unit rates): ACT does
    relu(psum + c[m,i]) with free-axis accum_out; DVE does
    max(psum, -c[m,i]) with add-reduce accum_out (tensor_scalar: with
    accum_out, op1 is the reduction op), whose missing +N*c is restored by
    two strided corrections per (l, mt) on the DVE columns only.
  - accum_out columns land directly in sT[m, i]; sT is cast to fp16 for the
    K=256 msg_w2 matmul; h update, readout and classifier run in fp32.
"""

import numpy as np

import concourse.bacc as bacc
import concourse.bass as bass
import concourse.tile as tile
from concourse import mybir
from concourse.bass_utils import run_bass_kernel_spmd

F32 = mybir.dt.float32
F16 = mybir.dt.float16
OP = mybir.AluOpType
AF = mybir.ActivationFunctionType

B, N, D, HID, MSG, RO, L, C = 8, 256, 256, 128, 256, 128, 2, 2
NCORES = 8


def _ap(t, ap, offset=0):
    return bass.AP(tensor=t.tensor, offset=t.offset + offset, ap=ap)


def build_nc():
    nc = bacc.Bacc("TRN2")

    xt = nc.dram_tensor("xt", [128, 2 * N], F16, kind="ExternalInput")
    wpack = nc.dram_tensor("wpack", [128, 64 * N], F16, kind="ExternalInput")
    encw1t = nc.dram_tensor("encw1t", [128, 2 * HID], F16, kind="ExternalInput")
    encw2t = nc.dram_tensor("encw2t", [128, HID], F16, kind="ExternalInput")
    encb1 = nc.dram_tensor("encb1", [128, 1], F32, kind="ExternalInput")
    encb2 = nc.dram_tensor("encb2", [128, 1], F32, kind="ExternalInput")
    watc = nc.dram_tensor("watc", [128, L * MSG], F16, kind="ExternalInput")
    wbtc = nc.dram_tensor("wbtc", [128, L * MSG], F16, kind="ExternalInput")
    ww4 = nc.dram_tensor("ww4", [128, L * 2 * 4 * 128], F16, kind="ExternalInput")
    msgb1 = nc.dram_tensor("msgb1", [128, L * 2], F32, kind="ExternalInput")
    w2tc = nc.dram_tensor("w2tc", [128, L * MSG], F16, kind="ExternalInput")
    nb2 = nc.dram_tensor("nb2", [128, L], F32, kind="ExternalInput")
    row1t = nc.dram_tensor("row1t", [128, RO], F32, kind="ExternalInput")
    rob1 = nc.dram_tensor("rob1", [128, 1], F32, kind="ExternalInput")
    row2t = nc.dram_tensor("row2t", [128, RO], F32, kind="ExternalInput")
    rob2 = nc.dram_tensor("rob2", [128, 1], F32, kind="ExternalInput")
    clswt = nc.dram_tensor("clswt", [128, C], F32, kind="ExternalInput")
    clsb = nc.dram_tensor("clsb", [C, 1], F32, kind="ExternalInput")
    y = nc.dram_tensor("y", [C, 1], F32, kind="ExternalOutput")

    with tile.TileContext(nc) as tc:
        with (
            tc.tile_pool(name="consts", bufs=1) as consts,
            tc.tile_pool(name="work", bufs=2) as work,
            tc.tile_pool(name="mmp", bufs=2, space="PSUM") as mmp,
            tc.tile_pool(name="inner", bufs=6, space="PSUM") as innerp,
        ):
            dma = nc.sync.dma_start

            # ---- constant loads -------------------------------------------
            t_xt = consts.tile([128, 2 * N], F16)
            dma(out=t_xt[:, :], in_=xt[:, :])
            t_encw1t = consts.tile([128, 2 * HID], F16)
            dma(out=t_encw1t[:, :], in_=encw1t[:, :])
            t_encw2t = consts.tile([128, HID], F16)
            dma(out=t_encw2t[:, :], in_=encw2t[:, :])
            t_encb1 = consts.tile([128, 1], F32)
            dma(out=t_encb1[:, :], in_=encb1[:, :])
            t_encb2 = consts.tile([128, 1], F32)
            dma(out=t_encb2[:, :], in_=encb2[:, :])
            t_wat = consts.tile([128, L * MSG], F16)
            dma(out=t_wat[:, :], in_=watc[:, :])
            t_wbt = consts.tile([128, L * MSG], F16)
            dma(out=t_wbt[:, :], in_=wbtc[:, :])
            t_msgb1 = consts.tile([128, L * 2], F32)
            dma(out=t_msgb1[:, :], in_=msgb1[:, :])
            t_w2t = consts.tile([128, L * MSG], F16)
            dma(out=t_w2t[:, :], in_=w2tc[:, :])
            t_nb2 = consts.tile([128, L], F32)
            dma(out=t_nb2[:, :], in_=nb2[:, :])
            t_row1t = consts.tile([128, RO], F32)
            dma(out=t_row1t[:, :], in_=row1t[:, :])
            t_rob1 = consts.tile([128, 1], F32)
            dma(out=t_rob1[:, :], in_=rob1[:, :])
            t_row2t = consts.tile([128, RO], F32)
            dma(out=t_row2t[:, :], in_=row2t[:, :])
            t_rob2 = consts.tile([128, 1], F32)
            dma(out=t_rob2[:, :], in_=rob2[:, :])
            t_clswt = consts.tile([128, C], F32)
            dma(out=t_clswt[:, :], in_=clswt[:, :])
            t_clsb = consts.tile([C, 1], F32)
            dma(out=t_clsb[:, :], in_=clsb[:, :])


            # W packed dense: 8 column chunks, alternating HWDGE/SWDGE
            # queues so transfers overlap; first pairs need only chunk 0.
            t_wpack = consts.tile([128, 64 * N], F16)
            t_ww4 = consts.tile([128, L * 2 * 4 * 128], F16)
            dma(out=t_ww4[:, :], in_=ww4[:, :])
            CH = 64 * N // 8
            for c in range(8):
                dma(out=t_wpack[:, c * CH:(c + 1) * CH],
                    in_=wpack[:, c * CH:(c + 1) * CH])

            # zero tile for the DVE max() path
            t_zero = consts.tile([128, N], F32)
            nc.vector.memset(t_zero[:, :], 0.0)

            # ---- encoder ---------------------------------------------------
            pe1 = mmp.tile([128, N], F32, tag="mm", name="pe1")
            nc.tensor.matmul(pe1[:, :], t_encw1t[:, 0:128], t_xt[:, 0:N],
                             start=True, stop=False)
            nc.tensor.matmul(pe1[:, :], t_encw1t[:, 128:256], t_xt[:, N:2 * N],
                             start=False, stop=True)
            t_h1 = work.tile([128, N], F16, tag="h1")
            nc.scalar.activation(out=t_h1[:, :], in_=pe1[:, :], func=AF.Relu,
                                 bias=t_encb1[:, 0:1])
            pe2 = mmp.tile([128, N], F32, tag="mm", name="pe2")
            nc.tensor.matmul(pe2[:, :], t_encw2t[:, :], t_h1[:, :],
                             start=True, stop=True)
            t_h = work.tile([128, N], F32, tag="hT")
            nc.vector.tensor_scalar(out=t_h[:, :], in0=pe2[:, :],
                                    scalar1=t_encb2[:, 0:1], scalar2=None,
                                    op0=OP.add)
            t_h16 = work.tile([128, N], F16, tag="hT16")
            nc.vector.tensor_copy(t_h16[:, :], t_h[:, :])

            # ---- message-passing layers -----------------------------------
            for l in range(L):
                # aT (+ msg_b1) per m-tile: bias columns for the inner loop
                t_ab = []
                for mt in range(2):
                    pa = mmp.tile([128, N], F32, tag="mm", name="pa")
                    nc.tensor.matmul(
                        pa[:, :],
                        t_wat[:, l * MSG + mt * 128: l * MSG + (mt + 1) * 128],
                        t_h16[:, :], start=True, stop=True)
                    ab = work.tile([128, N], F32, tag=f"ab{mt}")
                    nc.vector.tensor_scalar(
                        out=ab[:, :], in0=pa[:, :],
                        scalar1=t_msgb1[:, (l * 2 + mt):(l * 2 + mt) + 1],
                        scalar2=None, op0=OP.add)
                    nab = work.tile([128, N], F32, tag=f"nab{mt}", name=f"nab{mt}")
                    nc.vector.tensor_scalar(
                        out=nab[:, :], in0=pa[:, :],
                        scalar1=t_msgb1[:, (l * 2 + mt):(l * 2 + mt) + 1],
                        scalar2=-1.0, op0=OP.add, op1=OP.mult)
                    t_ab.append((ab, nab))

                t_sp = work.tile([128, 2 * N], F32, tag="sTp", name="sTp")
                t_s = [t_sp[:, mt * N:(mt + 1) * N] for mt in range(2)]

                # hT repeated twice along free axis (F=512 -> two i's)
                h_ap = t_h16[:, :]
                h_rep = _ap(h_ap, [h_ap.ap[0], [0, 2], h_ap.ap[1]])

                for mt in range(2):
                    wb_sl = t_wbt[:, l * MSG + mt * 128: l * MSG + (mt + 1) * 128]
                    for p in range(N // 2):
                        i0 = 2 * p
                        G = i0 // 64
                        blk = ((l * 2 + mt) * 4 + G) * 128
                        ofs = (i0 % 64) * N
                        ps = innerp.tile([128, 2 * N], F32)
                        nc.tensor.matmul(ps[:, :], wb_sl, h_rep,
                                         start=True, stop=False)
                        nc.tensor.matmul(
                            ps[:, :],
                            t_ww4[:, blk:blk + 128],
                            t_wpack[:, ofs:ofs + 2 * N],
                            start=False, stop=True)
                        for half in range(2):
                            i = i0 + half
                            sl = ps[:, half * N:(half + 1) * N]
                            if p % 2 == 0 and p % 16 != 0:
                                nc.scalar.activation(
                                    out=sl, in_=sl, func=AF.Relu,
                                    bias=t_ab[mt][0][:, i:i + 1],
                                    accum_out=t_sp[:, mt * N + i:mt * N + i + 1])
                            else:
                                # accum = sum_j max(pre-c, -c) = s - N*c;
                                # fixed after the loop on the DVE columns
                                nc.vector.tensor_scalar(
                                    out=sl, in0=sl,
                                    scalar1=t_ab[mt][1][:, i:i + 1],
                                    scalar2=None, op0=OP.max, op1=OP.add,
                                    accum_out=t_sp[:, mt * N + i:mt * N + i + 1])

                # message MLP second linear + relu + residual
                t_s16 = [work.tile([128, N], F16, tag=f"sT16_{mt}", name=f"sT16_{mt}")
                         for mt in range(2)]
                for mt in range(2):
                    base = t_sp[:, mt * N:(mt + 1) * N]
                    nabt = t_ab[mt][1][:, :]
                    # DVE pairs: p==0 (mod 16) -> cols 0..3 per 32; p odd ->
                    # cols 4j+2,4j+3 for j=1..7 per 32
                    for fpat, off in (([[4, N // 4], [1, 2]], 2),
                                      ([[32, N // 32], [1, 2]], 0)):
                        dcols = _ap(base, [base.ap[0]] + fpat, offset=off)
                        nabd = _ap(nabt, [nabt.ap[0]] + fpat, offset=off)
                        nc.vector.scalar_tensor_tensor(
                            out=dcols, in0=nabd, scalar=-float(N), in1=dcols,
                            op0=OP.mult, op1=OP.add)
                    nc.vector.tensor_copy(t_s16[mt][:, :],
                                          t_sp[:, mt * N:(mt + 1) * N])
                pm = mmp.tile([128, N], F32, tag="mm", name="pm")
                nc.tensor.matmul(pm[:, :], t_w2t[:, (l * 2) * 128:(l * 2 + 1) * 128],
                                 t_s16[0][:, :], start=True, stop=False)
                nc.tensor.matmul(pm[:, :], t_w2t[:, (l * 2 + 1) * 128:(l * 2 + 2) * 128],
                                 t_s16[1][:, :], start=False, stop=True)
                t_md = work.tile([128, N], F32, tag="md")
                nc.scalar.activation(out=t_md[:, :], in_=pm[:, :], func=AF.Relu,
                                     bias=t_nb2[:, l:l + 1])
                t_hn = work.tile([128, N], F32, tag="hT")
                nc.vector.tensor_tensor(out=t_hn[:, :], in0=t_h[:, :],
                                        in1=t_md[:, :], op=OP.add)
                t_h = t_hn
                if l + 1 < L:
                    t_h16 = work.tile([128, N], F16, tag="hT16", name="h16n")
                    nc.vector.tensor_copy(t_h16[:, :], t_h[:, :])

            # ---- readout + classifier -------------------------------------
            t_scr = work.tile([128, N], F32, tag="scr")
            t_z = work.tile([128, 1], F32, tag="zcol")
            nc.scalar.activation(out=t_scr[:, :], in_=t_h[:, :], func=AF.Copy,
                                 accum_out=t_z[:, 0:1])
            pr1 = mmp.tile([128, 1], F32, tag="mm", name="pr1")
            nc.tensor.matmul(pr1[:, :], t_row1t[:, :], t_z[:, 0:1],
                             start=True, stop=True)
            t_r1 = work.tile([128, 1], F32, tag="r1")
            nc.scalar.activation(out=t_r1[:, :], in_=pr1[:, :], func=AF.Relu,
                                 bias=t_rob1[:, 0:1])
            pr2 = mmp.tile([128, 1], F32, tag="mm", name="pr2")
            nc.tensor.matmul(pr2[:, :], t_row2t[:, :], t_r1[:, 0:1],
                             start=True, stop=True)
            t_zf = work.tile([128, 1], F32, tag="zf")
            nc.vector.scalar_tensor_tensor(out=t_zf[:, :], in0=pr2[:, :],
                                           scalar=t_rob2[:, 0:1],
                                           in1=t_z[:, 0:1],
                                           op0=OP.add, op1=OP.add)
            py = mmp.tile([C, 1], F32, tag="mm", name="py")
            nc.tensor.matmul(py[:, :], t_clswt[:, :], t_zf[:, 0:1],
                             start=True, stop=True)
            t_y = work.tile([C, 1], F32, tag="ysb")
            nc.vector.tensor_scalar(out=t_y[:, :], in0=py[:, :],
                                    scalar1=t_clsb[:, 0:1], scalar2=None,
                                    op0=OP.add)
            dma(out=y[:, :], in_=t_y[:, :])

    nc.finalize()
    return nc


def _prep_shared(params):
    """Host-side layout prep of the (tiny) shared parameter tensors."""
    f = np.float32
    enc_w1, enc_b1, enc_w2, enc_b2 = (params["enc_w1"], params["enc_b1"],
                                      params["enc_w2"], params["enc_b2"])
    msg_w1, msg_b1, msg_w2, msg_b2 = (params["msg_w1"], params["msg_b1"],
                                      params["msg_w2"], params["msg_b2"])
    ro_w1, ro_b1, ro_w2, ro_b2 = (params["ro_w1"], params["ro_b1"],
                                  params["ro_w2"], params["ro_b2"])
    cls_w, cls_b = params["cls_w"], params["cls_b"]

    w1t = np.ascontiguousarray(enc_w1.T)          # [D, HID]
    encw1t = np.concatenate([w1t[:128], w1t[128:]], axis=1)  # [128, 256]
    h = np.float16

    watc = np.zeros((128, L * MSG), h)
    wbtc = np.zeros((128, L * MSG), h)
    ww4 = np.zeros((128, L * 2 * 4 * 128), h)
    msgb1c = np.zeros((128, L * 2), f)
    w2tc = np.zeros((128, L * MSG), h)
    nb2c = np.zeros((128, L), f)
    for l in range(L):
        wa = msg_w1[l][:, :HID]                   # [MSG, HID]
        wb = msg_w1[l][:, HID:2 * HID]
        ww = msg_w1[l][:, -1]                     # [MSG]
        watc[:, l * MSG:(l + 1) * MSG] = wa.T
        wbtc[:, l * MSG:(l + 1) * MSG] = wb.T

        w2t = msg_w2[l].T                         # [MSG, HID]
        for mt in range(2):
            msgb1c[:, l * 2 + mt] = msg_b1[l][mt * 128:(mt + 1) * 128]
            for G in range(4):
                blk = ((l * 2 + mt) * 4 + G) * 128
                ww4[32 * G, blk:blk + 128] = ww[mt * 128:(mt + 1) * 128]
            w2tc[:, (l * 2 + mt) * 128:(l * 2 + mt + 1) * 128] = \
                w2t[mt * 128:(mt + 1) * 128]
        nb2c[:, l] = np.float32(N) * msg_b2[l]

    return {
        "encw1t": np.ascontiguousarray(encw1t, h),
        "encw2t": np.ascontiguousarray(enc_w2.T, h),
        "encb1": np.ascontiguousarray(enc_b1.reshape(128, 1), f),
        "encb2": np.ascontiguousarray(enc_b2.reshape(128, 1), f),
        "watc": watc, "wbtc": wbtc, "ww4": ww4, "msgb1": msgb1c,
        "w2tc": w2tc, "nb2": nb2c,
        "row1t": np.ascontiguousarray(ro_w1.T, f),
        "rob1": np.ascontiguousarray(ro_b1.reshape(RO, 1), f),
        "row2t": np.ascontiguousarray(ro_w2.T, f),
        "rob2": np.ascontiguousarray(ro_b2.reshape(RO, 1), f),
        "clswt": np.ascontiguousarray(cls_w.T, f),
        "clsb": np.ascontiguousarray(cls_b.reshape(C, 1), f),
    }


_NC_CACHE = {}


def _get_nc():
    if "nc" not in _NC_CACHE:
        _NC_CACHE["nc"] = build_nc()
    return _NC_CACHE["nc"]


def run(inputs, trace=False):
    inputs = {k: np.asarray(v, dtype=np.float32) for k, v in inputs.items()}
    shared = _prep_shared(inputs)
    W, X = inputs["W"], inputs["X"]
    in_maps = []
    for b in range(NCORES):
        xt = np.ascontiguousarray(X[b].T)         # [D, N]
        m = dict(shared)
        m["xt"] = np.ascontiguousarray(
            np.concatenate([xt[:128], xt[128:]], axis=1), np.float16)
        base = np.float16(W[b]).reshape(4, 64 * N)
        m["wpack"] = np.ascontiguousarray(np.repeat(base, 32, axis=0))
        in_maps.append(m)
    nc = _get_nc()
    res = run_bass_kernel_spmd(nc, in_maps, core_ids=list(range(NCORES)),
                               trace=trace)
    out = np.stack([res.results[b]["y"].reshape(C) for b in range(NCORES)])
    return np.ascontiguousarray(out, np.float32), res


def kernel(**inputs):
    out, _ = run(inputs)
    return out
